# revision 8
# baseline (speedup 1.0000x reference)
"""Trainium2 Bass kernel for nn_Loss_17695265260053 (retrieval_knn).

Computes, for B=16 batches of N=2048 3-D points:
  sym[b]  = mean_n min_m ||pred[b,n] - targ[b,m]||      (Chamfer / ADD-S)
  asym[b] = mean_n ||pred[b,n] - targ[b,n]||            (ADD)
  loss    = mean_b (flag[b]*sym[b] + (1-flag[b])*asym[b])

Sharding: data-parallel over batch, 2 batches per core on 8 cores; each
core emits one partial sum, the host sums partials and divides by B.

Per-core algorithm (per batch): targets are split host-side into two
halves A/B paired element-wise, and the PE computes per 128-pred tile
  s(n,j) = d2(n,Aj) + d2(n,Bj)      -> psS tile [128,1024] (2 banks)
  u(n,j) = d2(n,Aj) - d2(n,Bj)      -> psU tile [128,1024] (2 banks)
in K=16 fp16 matmuls (hi/lo error-free splits of the pair sums/diffs,
|p|^2 and |t|^2 folded in as extra rows; rows zero-padded to K=128 so
the full PE array is active -- the HAM clock governor only un-throttles
the PE to 2.4 GHz when all row-groups look busy; K=13 stays at 1.2).
Then min(d2_A, d2_B) = (s - |u|)/2, joined inside the PE:
  ScalarE: AB = |u| -> fp16 SBUF          (one PSUM-port read of u)
  PE:      psS += (-I).T @ AB             (accumulating join matmul)
  VectorE: tensor_scalar min-accum psS    (one PSUM-port read of s-|u|)
so every PSUM value crosses a PSUM read port exactly once and the DVE
does a single fused pass per tile. The join matmul is emitted one tile
behind (software pipelining) so the in-order PE queue never waits on
ScalarE. Epilogue: clamp, Sqrt(0.5x), sum-reduce; a ones-matmul reduces
across partitions; the sym_flag blend happens on [1,x] lanes.
"""

import sys

for _p in ("/opt/trn_rl_repo", "/opt/pypackages"):
    if _p not in sys.path:
        sys.path.insert(0, _p)

import numpy as np

import concourse.bass as bass
import concourse.tile as tile
from concourse import bacc, mybir

N_CORES = 8
B, N, D = 16, 2048, 3
BPC = B // N_CORES          # batches per core
NT = N // 128               # 16 pred tiles of 128 points
NP = 1024                   # target pairs per batch
KK = 128                    # contraction rows (16 used, zero-padded)
F32 = mybir.dt.float32
F16 = mybir.dt.float16
EPS = 1e-12
BIG = 1.0e30
Alu = mybir.AluOpType
Act = mybir.ActivationFunctionType
N_WARM_MM = 11              # full-array fp16 junk matmuls: ~6.5us HAM ramp


def build_loss_body(nc, tc, predt_d, targt_d, prednat_d, targnat_d,
                    negi_d, flag_d, out_d):
    """Emit the per-core program.
    predt_d:   [BPC, 128, N] f16 - rows [ph;ph;pl;pl;p2h;p2l;1;1] + zeros
    targt_d:   [BPC, 128, N] f16 - cols 0:1024 s-rows, 1024:2048 u-rows
    prednat_d: [BPC, 128, 48] f32 - tiled natural pred ([q, 3t+d] = pt 128t+q)
    targnat_d: [BPC, 128, 48] f32 - tiled natural target
    negi_d:    [128, 128] f16 - minus identity (join matmul weights)
    flag_d: [1, BPC]; out_d: [1, 1]."""
    with (
        tc.tile_pool(name="io", bufs=2) as io,
        tc.tile_pool(name="pre", bufs=2) as pre,
        tc.tile_pool(name="rhs", bufs=2) as rhsp,
        tc.tile_pool(name="ab", bufs=2) as abp,
        tc.tile_pool(name="acc", bufs=1) as accp,
        tc.tile_pool(name="psS", bufs=2, space="PSUM") as psS,
        tc.tile_pool(name="psU", bufs=2, space="PSUM") as psU,
    ):
        # per-core accumulators / constants
        SSUM = accp.tile([128, 2 * BPC], F32)   # cols: sym0, asym0, sym1, asym1
        ONES = accp.tile([128, 1], F32)
        nc.vector.memset(ONES[:], 1.0 / N)      # folds the 1/N mean into the reduce
        FL = accp.tile([1, BPC], F32)
        nc.sync.dma_start(FL[:], flag_d[:])
        NEGI = accp.tile([128, 128], F16)
        nc.sync.dma_start(NEGI[:], negi_d[:])

        # HAM warm-up: full-array junk matmuls keep all 128 PE rows active
        # through the activity window while the input DMAs land.
        JUNK = accp.tile([128, 512], F16)
        nc.vector.memset(JUNK[:], 0.0)
        wps = psS.tile([128, 512], F32, tag="ps")
        for _ in range(N_WARM_MM):
            nc.tensor.matmul(wps[:], JUNK[:, 0:128], JUNK[:], start=True,
                             stop=True)
        # dummy activations: pull the ACT table loads off the critical path
        ACTW = accp.tile([1, 3], F32)
        nc.scalar.activation(ACTW[:, 0:1], ONES[0:1, 0:1], Act.Square)
        nc.scalar.activation(ACTW[:, 1:2], ONES[0:1, 0:1], Act.Abs)
        nc.scalar.activation(ACTW[:, 2:3], ONES[0:1, 0:1], Act.Sqrt)

        # scratch main-out for the fused min-reduce (values never read)
        DUMPV = accp.tile([128, NP], F32)

        for b in range(BPC):
            # ---- loads ------------------------------------------------
            P4 = io.tile([128, NT * 3], F32, tag="P4")
            nc.sync.dma_start(P4[:], prednat_d[b])
            T4 = io.tile([128, NT * 3], F32, tag="T4")
            nc.sync.dma_start(T4[:], targnat_d[b])
            LT = rhsp.tile([KK, N], F16, tag="LT")
            nc.sync.dma_start(LT[:], predt_d[b])
            RT = rhsp.tile([KK, N], F16, tag="RT")
            nc.sync.dma_start(RT[:], targt_d[b])

            # ---- asym (ADD) branch on natural layout -------------------
            ADIF = pre.tile([128, NT * 3], F32, tag="adif")
            nc.vector.tensor_sub(ADIF[:], P4[:], T4[:])
            ASQ = pre.tile([128, NT * 3], F32, tag="asq")
            nc.scalar.activation(ASQ[:], ADIF[:], Act.Square)
            av = ASQ.rearrange("q (t d) -> q t d", d=3)
            AD2 = pre.tile([128, NT], F32, tag="ad2")
            nc.vector.tensor_add(AD2[:], av[:, :, 0], av[:, :, 1])
            nc.vector.tensor_add(AD2[:], AD2[:], av[:, :, 2])
            ASQR = pre.tile([128, NT], F32, tag="asqr")
            nc.scalar.activation(ASQR[:], AD2[:], Act.Sqrt)
            nc.vector.reduce_sum(
                SSUM[:, 2 * b + 1 : 2 * b + 2], ASQR[:], axis=mybir.AxisListType.X
            )

            # ---- main loop: s/u matmuls, |u| cast, PE join, min-accum --
            MINS = pre.tile([128, NT], F32, tag="mins")
            pend = None   # (pss, AB, a) awaiting join + reduce

            def join_and_reduce(pss, AB, a):
                # PE: psS += (-I).T @ AB  (accumulate onto the s values)
                for c in range(2):
                    nc.tensor.matmul(
                        pss[:, 512 * c : 512 * (c + 1)],
                        NEGI[:],
                        AB[:, 512 * c : 512 * (c + 1)],
                        start=False,
                        stop=True,
                    )
                # DVE: min over pairs of (s - |u|) in one fused pass
                nc.vector.tensor_scalar(
                    DUMPV[:], pss[:], 0.0, None,
                    op0=Alu.add, op1=Alu.min, accum_out=MINS[:, a : a + 1],
                )

            for a in range(NT):
                lhs = LT[:, 128 * a : 128 * (a + 1)]
                psu = psU.tile([128, NP], F32, tag="pu")
                if a == 0:
                    # 1-col "toucher" ladder: spread the batch-boundary waits
                    # (psum WAR/WAW, LT DMA, RT DMA) over cheap matmuls so no
                    # LDWEIGHTS exceeds its sync-wait budget.
                    nc.tensor.matmul(
                        psu[0:1, 0:1], ONES[:], ONES[:], start=True, stop=True
                    )
                    nc.tensor.matmul(
                        psu[0:1, 1:2], LT[:, 0:1], LT[:, 0:1], start=True,
                        stop=True,
                    )
                    nc.tensor.matmul(
                        psu[0:1, 2:3], RT[:, 0:1], RT[:, 0:1], start=True,
                        stop=True,
                    )
                for c in range(2):
                    nc.tensor.matmul(
                        psu[:, 512 * c : 512 * (c + 1)],
                        lhs,
                        RT[:, NP + 512 * c : NP + 512 * (c + 1)],
                        start=True,
                        stop=True,
                    )
                pss = psS.tile([128, NP], F32, tag="ps")
                for c in range(2):
                    nc.tensor.matmul(
                        pss[:, 512 * c : 512 * (c + 1)],
                        lhs,
                        RT[:, 512 * c : 512 * (c + 1)],
                        start=True,
                        stop=False,
                    )
                # ScalarE: AB = |u| as fp16 (frees the u tile)
                AB = abp.tile([128, NP], F16, tag="ab")
                nc.scalar.activation(AB[:], psu[:], Act.Abs)
                # join one tile behind so the in-order PE never waits on ScalarE
                if pend is not None:
                    join_and_reduce(*pend)
                pend = (pss, AB, a)
            join_and_reduce(*pend)

            # ---- epilogue: clamp, sqrt(0.5x), row-sum ------------------
            # MINS holds 2*min-d2; the 0.5 folds into the Sqrt scale.
            D2M = pre.tile([128, NT], F32, tag="d2m")
            nc.vector.tensor_scalar_max(D2M[:], MINS[:], 2.0 * EPS)
            DSQ = pre.tile([128, NT], F32, tag="dsq")
            nc.scalar.activation(DSQ[:], D2M[:], Act.Sqrt, scale=0.5)
            nc.vector.reduce_sum(
                SSUM[:, 2 * b : 2 * b + 1], DSQ[:], axis=mybir.AxisListType.X
            )

        # ---- final: partition reduce + flag blend ----------------------
        FPS = psS.tile([1, 2 * BPC], F32, tag="ps")
        nc.tensor.matmul(FPS[:], ONES[:], SSUM[:], start=True, stop=True)
        FSB = accp.tile([1, 2 * BPC], F32)
        nc.vector.tensor_copy(FSB[:], FPS[:])
        fv = FSB.rearrange("p (b k) -> p b k", k=2)  # k: 0 = sym, 1 = asym
        T0 = accp.tile([1, BPC], F32)
        nc.vector.tensor_sub(T0[:], fv[:, :, 0], fv[:, :, 1])
        nc.vector.tensor_mul(T0[:], T0[:], FL[:])
        nc.vector.tensor_add(T0[:], T0[:], fv[:, :, 1])
        OUT = accp.tile([1, 1], F32)
        nc.vector.reduce_sum(OUT[:], T0[:], axis=mybir.AxisListType.X)
        nc.sync.dma_start(out_d[:], OUT[:])


def build_core_program():
    """Build the single-core Bass program (same program runs SPMD on all 8)."""
    nc = bacc.Bacc("TRN2", target_bir_lowering=False, debug=False)
    predt_d = nc.dram_tensor("predt", [BPC, KK, N], F16, kind="ExternalInput")
    targt_d = nc.dram_tensor("targt", [BPC, KK, N], F16, kind="ExternalInput")
    prednat_d = nc.dram_tensor("prednat", [BPC, 128, NT * 3], F32, kind="ExternalInput")
    targnat_d = nc.dram_tensor("targnat", [BPC, 128, NT * 3], F32, kind="ExternalInput")
    negi_d = nc.dram_tensor("negi", [128, 128], F16, kind="ExternalInput")
    flag_d = nc.dram_tensor("flag", [1, BPC], F32, kind="ExternalInput")
    out_d = nc.dram_tensor("out", [1, 1], F32, kind="ExternalOutput")
    with tile.TileContext(nc) as tc:
        build_loss_body(nc, tc, predt_d.ap(), targt_d.ap(),
                        prednat_d.ap(), targnat_d.ap(), negi_d.ap(),
                        flag_d.ap(), out_d.ap())
    nc.compile()
    return nc


def _hilo(x):
    """Error-free fp16 hi/lo split of an fp32 array."""
    h = x.astype(np.float16)
    l = (x - h.astype(np.float32)).astype(np.float16)
    return h, l


def _morton_order(pts):
    """Sort order of [N,3] points along a 30-bit Morton (Z-order) curve."""
    q = np.clip(((pts + 5.0) * (1024.0 / 10.0)).astype(np.int64), 0, 1023)
    code = np.zeros(len(pts), np.int64)
    for i in range(10):
        for d in range(3):
            code |= ((q[:, d] >> i) & 1) << (3 * i + d)
    return np.argsort(code, kind="stable")


def host_inputs(pred_points, targ_points, sym_flag):
    """Host-side input formatting (shard + layout/precision split only)."""
    pred = np.asarray(pred_points, dtype=np.float32)
    targ = np.asarray(targ_points, dtype=np.float32)

    # lhs rows: [ph(3); ph(3); pl(3); pl(3); p2h; p2l; 1; 1] zero-pad to 128
    pt = (-2.0 * pred).transpose(0, 2, 1)             # [B, 3, N], exact scaling
    ph, pl = _hilo(pt)
    p2 = np.sum(pred * pred, axis=-1)[:, None, :]     # [B, 1, N]
    p2h, p2l = _hilo(p2)
    ones = np.ones((B, 1, N), np.float16)
    predt = np.zeros((B, KK, N), np.float16)
    predt[:, 0:16] = np.concatenate(
        [ph, ph, pl, pl, p2h, p2l, ones, ones], axis=1
    )

    # rhs rows per target pair j = (A_j, B_j): Morton-order adjacent targets
    # are paired so near-min pairs have small |u| = |d2_A - d2_B| -- the
    # fp16 rounding of |u| then lands far below the min-d2 scale.
    #   s-cols: [hi(tA+tB)(3); lo(.)(3); hi(.)(3); lo(.)(3); 2; 2;
    #            hi(|tA|^2+|tB|^2); lo(.)]
    #   u-cols: same with (tA-tB), 0, 0, |tA|^2-|tB|^2
    tord = np.empty_like(targ)
    for bi in range(B):
        tord[bi] = targ[bi][_morton_order(targ[bi])]
    tAn = tord[:, 0::2]                               # [B, 1024, 3]
    tBn = tord[:, 1::2]
    tA = tAn.transpose(0, 2, 1)                       # [B, 3, 1024]
    tB = tBn.transpose(0, 2, 1)
    t2A = np.sum(tAn ** 2, axis=-1)[:, None, :]
    t2B = np.sum(tBn ** 2, axis=-1)[:, None, :]
    tsh, tsl = _hilo(tA + tB)
    tdh, tdl = _hilo(tA - tB)
    t2sh, t2sl = _hilo(t2A + t2B)
    t2dh, t2dl = _hilo(t2A - t2B)
    twos = np.full((B, 1, NP), 2.0, np.float16)
    zeros = np.zeros((B, 1, NP), np.float16)
    srows = np.concatenate([tsh, tsl, tsh, tsl, twos, twos, t2sh, t2sl], axis=1)
    urows = np.concatenate([tdh, tdl, tdh, tdl, zeros, zeros, t2dh, t2dl], axis=1)
    targt = np.zeros((B, KK, N), np.float16)
    targt[:, 0:16, 0:NP] = srows
    targt[:, 0:16, NP:N] = urows

    negi = (-np.eye(128)).astype(np.float16)

    tiled = lambda x: np.ascontiguousarray(
        x.reshape(B, NT, 128, 3).transpose(0, 2, 1, 3).reshape(B, 128, NT * 3)
    )
    return (predt, targt, tiled(pred), tiled(targ), negi,
            np.asarray(sym_flag, dtype=np.float32))


def make_in_maps(pred_points, targ_points, sym_flag):
    predt, targt, prednat, targnat, negi, flags = host_inputs(
        pred_points, targ_points, sym_flag
    )
    in_maps = []
    for c in range(N_CORES):
        sl = slice(c * BPC, (c + 1) * BPC)
        in_maps.append(
            {
                "predt": np.ascontiguousarray(predt[sl]),
                "targt": np.ascontiguousarray(targt[sl]),
                "prednat": np.ascontiguousarray(prednat[sl]),
                "targnat": np.ascontiguousarray(targnat[sl]),
                "negi": negi,
                "flag": np.ascontiguousarray(flags[sl].reshape(1, BPC)),
            }
        )
    return in_maps


_NC_CACHE = None


def _get_nc():
    global _NC_CACHE
    if _NC_CACHE is None:
        _NC_CACHE = build_core_program()
    return _NC_CACHE


def run_spmd(pred_points, target_points, sym_flag, trace=False):
    from concourse.bass_utils import run_bass_kernel_spmd

    res = run_bass_kernel_spmd(
        _get_nc(),
        make_in_maps(pred_points, target_points, sym_flag),
        list(range(N_CORES)),
        trace=trace,
    )
    partials = [float(res.results[c]["out"][0, 0]) for c in range(N_CORES)]
    return np.float32(sum(partials) / B), res


def kernel(pred_points, target_points, sym_flag):
    out, _ = run_spmd(pred_points, target_points, sym_flag, trace=False)
    return np.asarray(out, dtype=np.float32)


# revision 14
# speedup vs baseline: 1.0500x; 1.0500x over previous
"""Trainium2 Bass kernel for nn_Loss_17695265260053 (retrieval_knn).

Computes, for B=16 batches of N=2048 3-D points:
  sym[b]  = mean_n min_m ||pred[b,n] - targ[b,m]||      (Chamfer / ADD-S)
  asym[b] = mean_n ||pred[b,n] - targ[b,n]||            (ADD)
  loss    = mean_b (flag[b]*sym[b] + (1-flag[b])*asym[b])

Sharding: data-parallel over batch, 2 batches per core on 8 cores; each
core emits one partial sum, the host sums partials and divides by B.

Per-core algorithm (per batch): targets are split host-side into two
halves A/B paired element-wise, and the PE computes per 128-pred tile
  s(n,j) = d2(n,Aj) + d2(n,Bj)      -> psS tile [128,1024] (2 banks)
  u(n,j) = d2(n,Aj) - d2(n,Bj)      -> psU tile [128,1024] (2 banks)
in K=16 fp16 matmuls (hi/lo error-free splits of the pair sums/diffs,
|p|^2 and |t|^2 folded in as extra rows; rows zero-padded to K=128 so
the full PE array is active -- the HAM clock governor only un-throttles
the PE to 2.4 GHz when all row-groups look busy; K=13 stays at 1.2).
Then min(d2_A, d2_B) = (s - |u|)/2, joined inside the PE:
  ScalarE: AB = |u| -> fp16 SBUF          (one PSUM-port read of u)
  PE:      psS += (-I).T @ AB             (accumulating join matmul)
  VectorE: tensor_scalar min-accum psS    (one PSUM-port read of s-|u|)
so every PSUM value crosses a PSUM read port exactly once and the DVE
does a single fused pass per tile. The join matmul is emitted one tile
behind (software pipelining) so the in-order PE queue never waits on
ScalarE. Epilogue: clamp, Sqrt(0.5x), sum-reduce; a ones-matmul reduces
across partitions; the sym_flag blend happens on [1,x] lanes.
"""

import sys

for _p in ("/opt/trn_rl_repo", "/opt/pypackages"):
    if _p not in sys.path:
        sys.path.insert(0, _p)

import numpy as np

import concourse.bass as bass
import concourse.tile as tile
from concourse import bacc, mybir

N_CORES = 8
B, N, D = 16, 2048, 3
BPC = B // N_CORES          # batches per core
NT = N // 128               # 16 pred tiles of 128 points
NP = 1024                   # target pairs per batch
KK = 128                    # contraction rows (16 used, zero-padded)
F32 = mybir.dt.float32
F16 = mybir.dt.float16
EPS = 1e-12
BIG = 1.0e30
Alu = mybir.AluOpType
Act = mybir.ActivationFunctionType
N_WARM_MM = 9               # full-array fp16 junk matmuls: ~6.5us HAM ramp


def build_loss_body(nc, tc, predt_d, targt_d, nat_d, negi_d, flag_d, out_d):
    """Emit the per-core program.
    predt_d: [BPC, 128, N] f16 - rows [ph;ph;pl;pl;p2h;p2l;1;1] + zeros
    targt_d: [BPC, 128, N] f16 - cols 0:1024 s-rows, 1024:2048 u-rows
    nat_d:   [BPC, 128, 96] f32 - tiled natural pred (cols 0:48) and
             target (cols 48:96), [q, 3t+d] = point 128t+q
    negi_d:  [128, 128] f16 - minus identity (join matmul weights)
    flag_d: [1, BPC]; out_d: [1, 1]."""
    with (
        tc.tile_pool(name="io", bufs=2) as io,
        tc.tile_pool(name="pre", bufs=2) as pre,
        tc.tile_pool(name="rhs", bufs=2) as rhsp,
        tc.tile_pool(name="ab", bufs=2) as abp,
        tc.tile_pool(name="acc", bufs=1) as accp,
        tc.tile_pool(name="psS", bufs=2, space="PSUM") as psS,
        tc.tile_pool(name="psU", bufs=2, space="PSUM") as psU,
    ):
        # ---- all input DMAs up front, big matmul operands first --------
        # (the sync queue serializes the ~0.6us triggers; LT/RT gate the
        # first real matmuls, so they go before everything else)
        LTs, RTs, NATs = [], [], []
        for b in range(BPC):
            LT = rhsp.tile([KK, N], F16, tag="LT")
            nc.sync.dma_start(LT[:], predt_d[b])
            RT = rhsp.tile([KK, N], F16, tag="RT")
            nc.sync.dma_start(RT[:], targt_d[b])
            LTs.append(LT)
            RTs.append(RT)
        for b in range(BPC):
            NAT = io.tile([128, 96], F32, tag="NAT")
            nc.sync.dma_start(NAT[:], nat_d[b])
            NATs.append(NAT)
        NEGI = accp.tile([128, 128], F16)
        nc.sync.dma_start(NEGI[:], negi_d[:])
        FL = accp.tile([1, BPC], F32)
        nc.sync.dma_start(FL[:], flag_d[:])

        # per-core accumulators / constants
        SSUM = accp.tile([128, 2 * BPC], F32)   # cols: sym0, asym0, sym1, asym1
        ONES = accp.tile([128, 1], F32)
        nc.vector.memset(ONES[:], 1.0 / N)      # folds the 1/N mean into the reduce

        # HAM warm-up: full-array junk matmuls keep all 128 PE rows active
        # through the activity window while the input DMAs land.
        JUNK = accp.tile([128, 512], F16)
        nc.vector.memset(JUNK[:], 0.0)
        wps = psS.tile([128, 512], F32, tag="ps")
        for _ in range(N_WARM_MM):
            nc.tensor.matmul(wps[:], JUNK[:, 0:128], JUNK[:], start=True,
                             stop=True)
        # dummy activations: pull the ACT table loads off the critical path
        ACTW = accp.tile([1, 3], F32)
        nc.scalar.activation(ACTW[:, 0:1], ONES[0:1, 0:1], Act.Square)
        nc.scalar.activation(ACTW[:, 1:2], ONES[0:1, 0:1], Act.Abs)
        nc.scalar.activation(ACTW[:, 2:3], ONES[0:1, 0:1], Act.Sqrt)

        # scratch main-out for the fused min-reduce (values never read)
        DUMPV = accp.tile([128, NP], F32)

        for b in range(BPC):
            LT, RT, NAT = LTs[b], RTs[b], NATs[b]
            P4 = NAT[:, 0:48]
            T4 = NAT[:, 48:96]

            # ---- asym (ADD) branch on natural layout -------------------
            ADIF = pre.tile([128, NT * 3], F32, tag="adif")
            nc.vector.tensor_sub(ADIF[:], P4, T4)
            ASQ = pre.tile([128, NT * 3], F32, tag="asq")
            nc.scalar.activation(ASQ[:], ADIF[:], Act.Square)
            av = ASQ.rearrange("q (t d) -> q t d", d=3)
            AD2 = pre.tile([128, NT], F32, tag="ad2")
            nc.vector.tensor_add(AD2[:], av[:, :, 0], av[:, :, 1])
            nc.vector.tensor_add(AD2[:], AD2[:], av[:, :, 2])
            ASQR = pre.tile([128, NT], F32, tag="asqr")
            nc.scalar.activation(ASQR[:], AD2[:], Act.Sqrt)
            nc.vector.reduce_sum(
                SSUM[:, 2 * b + 1 : 2 * b + 2], ASQR[:], axis=mybir.AxisListType.X
            )

            # ---- main loop: s/u matmuls, |u| cast, PE join, min-accum --
            MINS = pre.tile([128, NT], F32, tag="mins")
            pend = None   # (pss, AB, a) awaiting join + reduce

            def join_and_reduce(pss, AB, a):
                # PE: psS += (-I).T @ AB  (accumulate onto the s values)
                for c in range(2):
                    nc.tensor.matmul(
                        pss[:, 512 * c : 512 * (c + 1)],
                        NEGI[:],
                        AB[:, 512 * c : 512 * (c + 1)],
                        start=False,
                        stop=True,
                    )
                # DVE: min over pairs of (s - |u|) in one fused pass
                nc.vector.tensor_scalar(
                    DUMPV[:], pss[:], 0.0, None,
                    op0=Alu.add, op1=Alu.min, accum_out=MINS[:, a : a + 1],
                )

            for a in range(NT):
                lhs = LT[:, 128 * a : 128 * (a + 1)]
                psu = psU.tile([128, NP], F32, tag="pu")
                if a == 0:
                    # 1-col "toucher" ladder: spread the batch-boundary waits
                    # (psum WAR/WAW, LT DMA, RT DMA) over cheap matmuls so no
                    # LDWEIGHTS exceeds its sync-wait budget.
                    nc.tensor.matmul(
                        psu[0:1, 0:1], ONES[:], ONES[:], start=True, stop=True
                    )
                    nc.tensor.matmul(
                        psu[0:1, 1:2], LT[:, 0:1], LT[:, 0:1], start=True,
                        stop=True,
                    )
                    nc.tensor.matmul(
                        psu[0:1, 2:3], RT[:, 0:1], RT[:, 0:1], start=True,
                        stop=True,
                    )
                for c in range(2):
                    nc.tensor.matmul(
                        psu[:, 512 * c : 512 * (c + 1)],
                        lhs,
                        RT[:, NP + 512 * c : NP + 512 * (c + 1)],
                        start=True,
                        stop=True,
                    )
                pss = psS.tile([128, NP], F32, tag="ps")
                for c in range(2):
                    nc.tensor.matmul(
                        pss[:, 512 * c : 512 * (c + 1)],
                        lhs,
                        RT[:, 512 * c : 512 * (c + 1)],
                        start=True,
                        stop=False,
                    )
                # ScalarE: AB = |u| as fp16 (frees the u tile)
                AB = abp.tile([128, NP], F16, tag="ab")
                nc.scalar.activation(AB[:], psu[:], Act.Abs)
                # join one tile behind so the in-order PE never waits on ScalarE
                if pend is not None:
                    join_and_reduce(*pend)
                pend = (pss, AB, a)
            join_and_reduce(*pend)

            # ---- epilogue: clamp, sqrt(0.5x), row-sum ------------------
            # MINS holds 2*min-d2; the 0.5 folds into the Sqrt scale.
            D2M = pre.tile([128, NT], F32, tag="d2m")
            nc.vector.tensor_scalar_max(D2M[:], MINS[:], 2.0 * EPS)
            DSQ = pre.tile([128, NT], F32, tag="dsq")
            nc.scalar.activation(DSQ[:], D2M[:], Act.Sqrt, scale=0.5)
            nc.vector.reduce_sum(
                SSUM[:, 2 * b : 2 * b + 1], DSQ[:], axis=mybir.AxisListType.X
            )

        # ---- final: partition reduce + flag blend ----------------------
        FPS = psS.tile([1, 2 * BPC], F32, tag="ps")
        nc.tensor.matmul(FPS[:], ONES[:], SSUM[:], start=True, stop=True)
        FSB = accp.tile([1, 2 * BPC], F32)
        nc.vector.tensor_copy(FSB[:], FPS[:])
        fv = FSB.rearrange("p (b k) -> p b k", k=2)  # k: 0 = sym, 1 = asym
        T0 = accp.tile([1, BPC], F32)
        nc.vector.tensor_sub(T0[:], fv[:, :, 0], fv[:, :, 1])
        nc.vector.tensor_mul(T0[:], T0[:], FL[:])
        nc.vector.tensor_add(T0[:], T0[:], fv[:, :, 1])
        OUT = accp.tile([1, 1], F32)
        nc.vector.reduce_sum(OUT[:], T0[:], axis=mybir.AxisListType.X)
        nc.sync.dma_start(out_d[:], OUT[:])


def build_core_program():
    """Build the single-core Bass program (same program runs SPMD on all 8)."""
    nc = bacc.Bacc("TRN2", target_bir_lowering=False, debug=False)
    predt_d = nc.dram_tensor("predt", [BPC, KK, N], F16, kind="ExternalInput")
    targt_d = nc.dram_tensor("targt", [BPC, KK, N], F16, kind="ExternalInput")
    nat_d = nc.dram_tensor("nat", [BPC, 128, 96], F32, kind="ExternalInput")
    negi_d = nc.dram_tensor("negi", [128, 128], F16, kind="ExternalInput")
    flag_d = nc.dram_tensor("flag", [1, BPC], F32, kind="ExternalInput")
    out_d = nc.dram_tensor("out", [1, 1], F32, kind="ExternalOutput")
    with tile.TileContext(nc) as tc:
        build_loss_body(nc, tc, predt_d.ap(), targt_d.ap(), nat_d.ap(),
                        negi_d.ap(), flag_d.ap(), out_d.ap())
    nc.compile()
    return nc


def _hilo(x):
    """Error-free fp16 hi/lo split of an fp32 array."""
    h = x.astype(np.float16)
    l = (x - h.astype(np.float32)).astype(np.float16)
    return h, l


def _morton_order(pts):
    """Sort order of [N,3] points along a 30-bit Morton (Z-order) curve."""
    q = np.clip(((pts + 5.0) * (1024.0 / 10.0)).astype(np.int64), 0, 1023)
    code = np.zeros(len(pts), np.int64)
    for i in range(10):
        for d in range(3):
            code |= ((q[:, d] >> i) & 1) << (3 * i + d)
    return np.argsort(code, kind="stable")


def host_inputs(pred_points, targ_points, sym_flag):
    """Host-side input formatting (shard + layout/precision split only)."""
    pred = np.asarray(pred_points, dtype=np.float32)
    targ = np.asarray(targ_points, dtype=np.float32)

    # lhs rows: [ph(3); ph(3); pl(3); pl(3); p2h; p2l; 1; 1] zero-pad to 128
    pt = (-2.0 * pred).transpose(0, 2, 1)             # [B, 3, N], exact scaling
    ph, pl = _hilo(pt)
    p2 = np.sum(pred * pred, axis=-1)[:, None, :]     # [B, 1, N]
    p2h, p2l = _hilo(p2)
    ones = np.ones((B, 1, N), np.float16)
    predt = np.zeros((B, KK, N), np.float16)
    predt[:, 0:16] = np.concatenate(
        [ph, ph, pl, pl, p2h, p2l, ones, ones], axis=1
    )

    # rhs rows per target pair j = (A_j, B_j): Morton-order adjacent targets
    # are paired so near-min pairs have small |u| = |d2_A - d2_B| -- the
    # fp16 rounding of |u| then lands far below the min-d2 scale.
    #   s-cols: [hi(tA+tB)(3); lo(.)(3); hi(.)(3); lo(.)(3); 2; 2;
    #            hi(|tA|^2+|tB|^2); lo(.)]
    #   u-cols: same with (tA-tB), 0, 0, |tA|^2-|tB|^2
    tord = np.empty_like(targ)
    for bi in range(B):
        tord[bi] = targ[bi][_morton_order(targ[bi])]
    tAn = tord[:, 0::2]                               # [B, 1024, 3]
    tBn = tord[:, 1::2]
    tA = tAn.transpose(0, 2, 1)                       # [B, 3, 1024]
    tB = tBn.transpose(0, 2, 1)
    t2A = np.sum(tAn ** 2, axis=-1)[:, None, :]
    t2B = np.sum(tBn ** 2, axis=-1)[:, None, :]
    tsh, tsl = _hilo(tA + tB)
    tdh, tdl = _hilo(tA - tB)
    t2sh, t2sl = _hilo(t2A + t2B)
    t2dh, t2dl = _hilo(t2A - t2B)
    twos = np.full((B, 1, NP), 2.0, np.float16)
    zeros = np.zeros((B, 1, NP), np.float16)
    srows = np.concatenate([tsh, tsl, tsh, tsl, twos, twos, t2sh, t2sl], axis=1)
    urows = np.concatenate([tdh, tdl, tdh, tdl, zeros, zeros, t2dh, t2dl], axis=1)
    targt = np.zeros((B, KK, N), np.float16)
    targt[:, 0:16, 0:NP] = srows
    targt[:, 0:16, NP:N] = urows

    negi = (-np.eye(128)).astype(np.float16)

    tiled = lambda x: np.ascontiguousarray(
        x.reshape(B, NT, 128, 3).transpose(0, 2, 1, 3).reshape(B, 128, NT * 3)
    )
    nat = np.concatenate([tiled(pred), tiled(targ)], axis=2)  # [B, 128, 96]
    return (predt, targt, nat, negi,
            np.asarray(sym_flag, dtype=np.float32))


def make_in_maps(pred_points, targ_points, sym_flag):
    predt, targt, nat, negi, flags = host_inputs(
        pred_points, targ_points, sym_flag
    )
    in_maps = []
    for c in range(N_CORES):
        sl = slice(c * BPC, (c + 1) * BPC)
        in_maps.append(
            {
                "predt": np.ascontiguousarray(predt[sl]),
                "targt": np.ascontiguousarray(targt[sl]),
                "nat": np.ascontiguousarray(nat[sl]),
                "negi": negi,
                "flag": np.ascontiguousarray(flags[sl].reshape(1, BPC)),
            }
        )
    return in_maps


_NC_CACHE = None


def _get_nc():
    global _NC_CACHE
    if _NC_CACHE is None:
        _NC_CACHE = build_core_program()
    return _NC_CACHE


def run_spmd(pred_points, target_points, sym_flag, trace=False):
    from concourse.bass_utils import run_bass_kernel_spmd

    res = run_bass_kernel_spmd(
        _get_nc(),
        make_in_maps(pred_points, target_points, sym_flag),
        list(range(N_CORES)),
        trace=trace,
    )
    partials = [float(res.results[c]["out"][0, 0]) for c in range(N_CORES)]
    return np.float32(sum(partials) / B), res


def kernel(pred_points, target_points, sym_flag):
    out, _ = run_spmd(pred_points, target_points, sym_flag, trace=False)
    return np.asarray(out, dtype=np.float32)
